# revision 11
# baseline (speedup 1.0000x reference)
"""PointTransformerLayer fused Bass/Tile kernel for Trainium2 (8 NeuronCores).

Reference computation (B=1, N=1024, D=64, H_POS=64, H_ATTN=256):
    q,k,v = x@Wq, x@Wk, x@Wv
    rpe   = relu((pos_i - pos_j)@pW1 + pb1)@pW2 + pb2
    sim   = relu((q_i - k_j + rpe)@aW1 + ab1)@aW2 + ab2
    attn  = softmax(sim, axis=j)            # per-channel
    out_i = sum_j attn * (v_j + rpe)

Algebraic restructuring (host-side precompute in numpy):
    h1[i,j]  = relu(a_i - a_j + pb1),  a = pos@pW1                 [64]
    t[i,j]   = Wpa.T h1[i,j] - Ka.T x_j + qab_i                    [256]
               Wpa = pW2@aW1, Ka = Wk@aW1, qab = x@Wq@aW1 + pb2@aW1 + ab1
    h2       = relu(t); sim = aW2.T h2  (ab2 dropped: softmax-invariant)
    vv[i,j]  = pW2.T h1[i,j] + Wv.T x_j  (pb2 added at the very end)
    out_i    = sum_j(exp(sim)*vv)/sum_j(exp(sim)) + pb2

Device mapping: query index i sharded 128-per-core. Per i the moving operand
is one SBUF tile [+/-h1_i (part 0:64); x.T (part 64:128)] x 1024 j, so each
of t / vv is a K=128 matmul and the -Ka.T x_j / Wv.T x_j terms ride along
free. Pairs of i stack the d=64 channel dim into 128 PSUM partitions for the
softmax stages via zero-padded stationaries (f32r matmuls cannot write a
nonzero dst partition offset). Softmax stats come from
activation(Exp, accum_out=S1) and tensor_tensor_reduce(E*vv -> S2).
h1 may be computed as -h1 = min(a_j - a_i - pb1, 0) on VectorE (sign folded
into the stationary), balancing ScalarE/VectorE load.
"""

import numpy as np

D = 64
HP = 64
HA = 256
NPT = 1024
NCORES = 8
IPC = NPT // NCORES
PAIRS = IPC // 2

MM_DTYPE = "float32r"

# per (mm_dtype): engine for h1 of par0/par1 and for relu of (par, half)
_CFG = {
    "float32r": {
        "h1": ("scalar", "vector"),
        "relu": {(0, 0): "scalar", (0, 1): "vector",
                 (1, 0): "scalar", (1, 1): "vector"},
    },
    "bfloat16": {
        "h1": ("vector", "vector"),
        "relu": {(0, 0): "scalar", (0, 1): "scalar",
                 (1, 0): "scalar", (1, 1): "vector"},
    },
}

_NC_CACHE = {}


def _np_dt(mm_dtype_name):
    from concourse import mybir
    return mybir.dt.np(getattr(mybir.dt, mm_dtype_name))


def _h1_signs(mm_dtype_name):
    # +1 when h1 computed on ScalarE (relu -> +h1), -1 on VectorE (min -> -h1)
    return tuple(+1 if e == "scalar" else -1
                 for e in _CFG[mm_dtype_name]["h1"])


def _build_nc(mm_dtype_name=MM_DTYPE, n_pairs=PAIRS, reps=1):
    import concourse.bacc as bacc
    import concourse.tile as tile
    from concourse import mybir

    f32 = mybir.dt.float32
    bf16 = mybir.dt.bfloat16
    mmdt = getattr(mybir.dt, mm_dtype_name)
    aptdt = bf16 if mm_dtype_name == "bfloat16" else f32
    AF = mybir.ActivationFunctionType
    ALU = mybir.AluOpType
    cfg = _CFG[mm_dtype_name]

    nc = bacc.Bacc("TRN2", target_bir_lowering=False, debug=False)

    apt_d = nc.dram_tensor("apt", [HP, NPT], aptdt, kind="ExternalInput").ap()
    xt_d = nc.dram_tensor("xt", [D, NPT], mmdt, kind="ExternalInput").ap()
    st_d = nc.dram_tensor("st", [128, 4 * 128], mmdt, kind="ExternalInput").ap()
    sv_d = nc.dram_tensor("sv", [128, 2 * 128], mmdt, kind="ExternalInput").ap()
    aw2_d = nc.dram_tensor("aw2", [128, 4 * 128], mmdt, kind="ExternalInput").ap()
    qab0_d = nc.dram_tensor("qab0", [128, IPC], f32, kind="ExternalInput").ap()
    qab1_d = nc.dram_tensor("qab1", [128, IPC], f32, kind="ExternalInput").ap()
    apb_d = nc.dram_tensor("apb", [HP, IPC], f32, kind="ExternalInput").ap()
    pb2_d = nc.dram_tensor("pb2", [128, 1], f32, kind="ExternalInput").ap()
    o_d = nc.dram_tensor("o", [128, n_pairs], f32, kind="ExternalOutput").ap()

    with tile.TileContext(nc) as tc:
        with (
            tc.tile_pool(name="singles", bufs=1) as sg,
            tc.tile_pool(name="h2p", bufs=4) as h2p,
            tc.tile_pool(name="ep", bufs=2) as ep,
            tc.tile_pool(name="jp", bufs=2) as jp,
            tc.tile_pool(name="ptp", bufs=2, space="PSUM") as ptp,
            tc.tile_pool(name="pvp", bufs=1, space="PSUM") as pvp,
            tc.tile_pool(name="psp", bufs=1, space="PSUM") as psp,
        ):
            APT = sg.tile([HP, NPT], aptdt, tag="apt")
            M = [sg.tile([128, NPT], mmdt, tag=f"m{k}", name=f"m{k}")
                 for k in range(2)]
            ST = sg.tile([128, 4 * 128], mmdt, tag="st")
            SV = sg.tile([128, 2 * 128], mmdt, tag="sv")
            AW2 = sg.tile([128, 4 * 128], mmdt, tag="aw2")
            QAB = [sg.tile([128, IPC], f32, tag=f"qab{k}", name=f"qab{k}")
                   for k in range(2)]
            APB = sg.tile([HP, IPC], f32, tag="apb")
            PB2 = sg.tile([128, 1], f32, tag="pb2")
            RS1 = sg.tile([128, n_pairs], f32, tag="rs1")
            RS2 = sg.tile([128, n_pairs], f32, tag="rs2")
            T1 = sg.tile([128, n_pairs], f32, tag="t1")
            OUTT = sg.tile([128, n_pairs], f32, tag="outt")

            nc.sync.dma_start(APT[:], apt_d[:])
            nc.sync.dma_start(M[0][HP:128, :], xt_d[:])
            nc.sync.dma_start(M[1][HP:128, :], xt_d[:])
            nc.sync.dma_start(ST[:], st_d[:])
            nc.sync.dma_start(SV[:], sv_d[:])
            nc.sync.dma_start(AW2[:], aw2_d[:])
            nc.sync.dma_start(QAB[0][:], qab0_d[:])
            nc.sync.dma_start(QAB[1][:], qab1_d[:])
            nc.sync.dma_start(APB[:], apb_d[:])
            nc.sync.dma_start(PB2[:], pb2_d[:])

            for p in [pp for _ in range(reps) for pp in range(n_pairs)]:
                PV = pvp.tile([128, NPT], f32, tag="pv")
                PS = psp.tile([128, NPT], f32, tag="ps")
                for par in range(2):
                    i = 2 * p + par
                    m = M[i % 2]
                    # h1 (or -h1) -> upper half of the moving tile
                    if cfg["h1"][par] == "scalar":
                        nc.scalar.activation(
                            m[0:HP, :], APT[:], AF.Relu,
                            bias=APB[:, i:i + 1], scale=-1.0,
                        )
                    else:
                        nc.vector.tensor_scalar(
                            m[0:HP, :], APT[:], APB[:, i:i + 1], 0.0,
                            op0=ALU.subtract, op1=ALU.min,
                        )
                    H2 = [h2p.tile([128, NPT], mmdt, tag="h2", name="h2")
                          for _ in range(2)]
                    for h in range(2):
                        PT = ptp.tile([128, NPT], f32, tag="pt")
                        blk = (2 * par + h) * 128
                        for jc in range(2):
                            nc.tensor.matmul(
                                PT[:, 512 * jc:512 * (jc + 1)],
                                ST[:, blk:blk + 128],
                                m[:, 512 * jc:512 * (jc + 1)],
                                start=True, stop=True,
                            )
                        # h2 = relu(t + qab_i)
                        if cfg["relu"][(par, h)] == "scalar":
                            nc.scalar.activation(
                                H2[h][:], PT[:], AF.Relu,
                                bias=QAB[h][:, i:i + 1], scale=1.0,
                            )
                        else:
                            nc.vector.tensor_scalar(
                                H2[h][:], PT[:], QAB[h][:, i:i + 1], 0.0,
                                op0=ALU.add, op1=ALU.max,
                            )
                    # vv accumulated over par via zero-padded stationaries
                    for jc in range(2):
                        nc.tensor.matmul(
                            PV[:, 512 * jc:512 * (jc + 1)],
                            SV[:, 128 * par:128 * (par + 1)],
                            m[:, 512 * jc:512 * (jc + 1)],
                            start=(par == 0), stop=(par == 1),
                            skip_group_check=True,
                        )
                    # sim accumulated over (par, kh)
                    for jc in range(2):
                        for kh in range(2):
                            nc.tensor.matmul(
                                PS[:, 512 * jc:512 * (jc + 1)],
                                AW2[:, (2 * par + kh) * 128:(2 * par + kh + 1) * 128],
                                H2[kh][:, 512 * jc:512 * (jc + 1)],
                                start=(par == 0 and kh == 0),
                                stop=(par == 1 and kh == 1),
                                skip_group_check=True,
                            )
                E = ep.tile([128, NPT], bf16, tag="e")
                nc.scalar.activation(
                    E[:], PS[:], AF.Exp, accum_out=RS1[:, p:p + 1],
                )
                J = jp.tile([128, NPT], bf16, tag="j")
                nc.vector.scalar_tensor_tensor(
                    out=J[:], in0=E[:], scalar=1.0, in1=PV[:],
                    op0=ALU.mult, op1=ALU.mult, accum_out=RS2[:, p:p + 1],
                )

            nc.vector.reciprocal(T1[:], RS1[:])
            nc.vector.tensor_mul(OUTT[:], RS2[:], T1[:])
            nc.vector.tensor_scalar(OUTT[:], OUTT[:], PB2[:, 0:1], None,
                                    op0=ALU.add)
            nc.sync.dma_start(o_d[:], OUTT[:])

    nc.compile()
    return nc


def _get_nc(mm_dtype_name=MM_DTYPE):
    if mm_dtype_name not in _NC_CACHE:
        _NC_CACHE[mm_dtype_name] = _build_nc(mm_dtype_name)
    return _NC_CACHE[mm_dtype_name]


def make_in_maps(x, pos, Wq, Wk, Wv, pW1, pb1, pW2, pb2, aW1, ab1, aW2, ab2,
                 mm_dtype_name=MM_DTYPE):
    f = np.float32
    mmnp = _np_dt(mm_dtype_name)
    aptnp = mmnp if mm_dtype_name == "bfloat16" else f
    sgn = _h1_signs(mm_dtype_name)

    x2 = np.ascontiguousarray(np.asarray(x, f)[0])        # [N, D]
    pos2 = np.ascontiguousarray(np.asarray(pos, f)[0])    # [N, 2]
    Wq, Wk, Wv = (np.asarray(a, f) for a in (Wq, Wk, Wv))
    pW1, pb1, pW2, pb2 = (np.asarray(a, f) for a in (pW1, pb1, pW2, pb2))
    aW1, ab1, aW2, ab2 = (np.asarray(a, f) for a in (aW1, ab1, aW2, ab2))

    a_pos = pos2 @ pW1                                    # [N, HP]
    apt = np.ascontiguousarray(a_pos.T)                   # [HP, N]
    xt = np.ascontiguousarray(x2.T)                       # [D, N]
    Wpa = pW2 @ aW1                                       # [HP, HA]
    Ka = Wk @ aW1                                         # [D, HA]

    # t stationaries: 4 blocks of [128 x 128], block = (par*2 + h)
    st = np.zeros((128, 4 * 128), f)
    for par in range(2):
        for h in range(2):
            blk = (2 * par + h) * 128
            st[0:HP, blk:blk + 128] = sgn[par] * Wpa[:, h * 128:(h + 1) * 128]
            st[HP:128, blk:blk + 128] = -Ka[:, h * 128:(h + 1) * 128]

    # vv stationaries: 2 blocks [128 x 128], par0 -> out cols 0:64, par1 -> 64:128
    sv = np.zeros((128, 2 * 128), f)
    for par in range(2):
        blk = par * 128
        sv[0:HP, blk + par * D: blk + par * D + D] = sgn[par] * pW2
        sv[HP:128, blk + par * D: blk + par * D + D] = Wv

    # sim stationaries: 4 blocks [128 x 128], block = (par*2 + kh)
    aw2 = np.zeros((128, 4 * 128), f)
    for par in range(2):
        for kh in range(2):
            blk = (2 * par + kh) * 128
            aw2[:, blk + par * D: blk + par * D + D] = aW2[kh * 128:(kh + 1) * 128, :]

    qab = x2 @ (Wq @ aW1) + (pb2 @ aW1 + ab1)             # [N, HA]
    apb = np.ascontiguousarray((a_pos + pb1).T)           # [HP, N]
    pb2p = np.ascontiguousarray(np.concatenate([pb2, pb2]).reshape(128, 1))

    in_maps = []
    for c in range(NCORES):
        sl = slice(c * IPC, (c + 1) * IPC)
        in_maps.append({
            "apt": apt.astype(aptnp),
            "xt": xt.astype(mmnp),
            "st": st.astype(mmnp),
            "sv": sv.astype(mmnp),
            "aw2": aw2.astype(mmnp),
            "qab0": np.ascontiguousarray(qab[sl, 0:128].T),
            "qab1": np.ascontiguousarray(qab[sl, 128:256].T),
            "apb": np.ascontiguousarray(apb[:, sl]),
            "pb2": pb2p,
        })
    return in_maps


def gather_out(results):
    outs = []
    for c in range(NCORES):
        o = results[c]["o"]                 # [128, PAIRS]
        A = o.reshape(2, D, PAIRS)          # [half, d, pair]
        outs.append(A.transpose(2, 0, 1).reshape(IPC, D))
    return np.concatenate(outs, axis=0)[None].astype(np.float32)


def kernel(**inputs):
    from concourse.bass_utils import run_bass_kernel_spmd

    nc = _get_nc()
    in_maps = make_in_maps(**inputs)
    res = run_bass_kernel_spmd(nc, in_maps, core_ids=list(range(NCORES)))
    return gather_out(res.results)
